# revision 26
# baseline (speedup 1.0000x reference)
"""ChebyNet (K=3, 2 layers) on 8 Trainium2 NeuronCores.

Strategy (v2):
  - Algebra: (A x) W == A (x W)  -> push features through weights first, so all
    4 SpMVs run at width 64. With A = -D^-1/2 Ahat D^-1/2, pre-scaling rows by
    dinv turns every SpMV into an unweighted gather+segment-sum.
  - Sharding: nodes globally sorted by degree, dealt round-robin to 8 cores
    (rank r -> core r%8, slot r//8); one static NEFF schedule fits all cores.
  - Tables are bf16, node-major [50176, 64] (= pair-major [25088, 128]).
    Gather fetches a 256B PAIR of nodes per edge with idx = row>>1 (fits
    int16 in a single window -> ~31% fewer descriptors than a 2-window
    split), then a DVE multiply+add with a {0,1} mask stream selects the
    wanted half exactly.
  - The gather descriptor generation on GpSimd (~5ns/edge, one Q7 pair per
    swdge queue) is the critical path. Batches rotate over 3 swdge queues
    (BUDGET=28 rounds/batch) so descriptor gen, DMA drain, and consume
    pipeline; AllGathers are bf16; layer-2 matmuls are fused into SpMV-2's
    epilogues; log_softmax runs as two bulk passes at the end.
"""
import os
import numpy as np

N, E, FIN, H, C = 50000, 800000, 100, 64, 40
NC = 8
P = 128
PC = 6272            # padded nodes per core (49*128; 6250 real)
T = PC // P          # 49 tiles
VP = NC * PC // 2    # 25088 table pairs
PAD_PAIR = 3125      # pair of core-0 pad slots (6250,6251) -> zero rows
BUDGET = 24          # staging budget in rounds per gather batch
AG_SPLIT = 40        # tiles [0,40) -> AG chunk A, [40,49) -> chunk B
ROWS_A = AG_SPLIT * P          # 5120
ROWS_B = PC - ROWS_A           # 1152


def _preprocess(x, edge_index):
    row = np.asarray(edge_index[0], dtype=np.int64)
    col = np.asarray(edge_index[1], dtype=np.int64)
    deg = np.bincount(row, minlength=N).astype(np.int64)
    order = np.argsort(-deg, kind="stable")        # rank -> node
    rank = np.empty(N, np.int64); rank[order] = np.arange(N)
    corer, posr = rank % NC, rank // NC

    ecore = corer[row]; epos = posr[row]
    pidx = (corer[col] * (PC // 2) + (posr[col] >> 1)).astype(np.int64)
    phal = (posr[col] & 1).astype(np.int64)

    cnt = np.zeros((NC, PC), np.int64)
    np.add.at(cnt, (ecore, epos), 1)
    DMAX = int(cnt.max())
    da = cnt.reshape(NC, T, P).max(axis=(0, 2)).astype(np.int64)  # shared

    ell = np.full((NC, PC, DMAX), PAD_PAIR, np.int64)
    ellh = np.zeros((NC, PC, DMAX), np.int64)
    for c in range(NC):
        m = ecore == c
        ep, pi, ph = epos[m], pidx[m], phal[m]
        o = np.argsort(ep, kind="stable")
        ep, pi, ph = ep[o], pi[o], ph[o]
        starts = np.searchsorted(ep, np.arange(PC))
        j = np.arange(len(ep)) - starts[ep]
        ell[c, ep, j] = pi
        ellh[c, ep, j] = ph

    # batches of whole tiles, bounded staging rounds
    batches = []
    cur, b = [], 0
    for t in range(T):
        r = int(da[t])
        if cur and b + r > BUDGET:
            batches.append(cur); cur, b = [], 0
        cur.append(t); b += r
    if cur:
        batches.append(cur)

    R = int(da.sum())
    # idx buffer: column layout = sequential tiles, 8 int16 cols per round;
    # idx k of a gather goes to [k%16, off + k//16]; replicated to 128 parts.
    def wrap(seq):
        return np.tile(seq.reshape(-1, 16).T.astype(np.int16), (8, 1))

    idxbuf = np.zeros((NC, 128, 8 * R), np.int16)
    # mask: per round 128 cols: [0:64]=left(1-h), [64:128]=right(h)
    mleft = np.zeros((NC, R, P), np.float32)
    off = 0
    for t in range(T):
        dt = int(da[t])
        for c in range(NC):
            seq = ell[c, t * P:(t + 1) * P, :dt].T.ravel()
            idxbuf[c, :, 8 * off:8 * (off + dt)] = wrap(seq)
        mleft[:, off:off + dt, :] = 1.0 - ellh[:, t * P:(t + 1) * P, :dt].transpose(0, 2, 1)
        off += dt

    import ml_dtypes
    maskbuf = np.empty((NC, P, 2 * R), ml_dtypes.bfloat16)
    maskbuf[:, :, :R] = mleft.transpose(0, 2, 1)
    maskbuf[:, :, R:] = 1.0 - mleft.transpose(0, 2, 1)

    deg_pt = np.zeros((NC, P, T), np.float32)
    xts = np.zeros((NC, FIN + 1, PC), ml_dtypes.bfloat16)
    for c in range(NC):
        nq = min(PC, (N - c + NC - 1) // NC)  # real nodes on this core (6250)
        nodes = order[np.arange(nq) * NC + c]
        q = np.arange(nq)
        deg_pt[c, q % P, q // P] = deg[nodes]
        xts[c, :FIN, :nq] = x[nodes].T
        xts[c, FIN, :nq] = 1.0
    return order, da, batches, idxbuf, maskbuf, deg_pt, xts


def _build(da, batches):
    import concourse.bacc as bacc
    import concourse.mybir as mybir
    import concourse.tile as tile
    from concourse.masks import make_identity
    f32 = mybir.dt.float32
    bf16 = mybir.dt.bfloat16

    R = int(da.sum())
    IC = 8 * R
    nc = bacc.Bacc("TRN2", target_bir_lowering=False, debug=False, num_devices=NC,
                   num_swdge_queues=4)
    xt_d = nc.dram_tensor("xt", [FIN + 1, PC], bf16, kind="ExternalInput")
    w1_d = nc.dram_tensor("w1p", [FIN + 1, 3 * H], bf16, kind="ExternalInput")
    w2_d = nc.dram_tensor("w2p", [H + 1, 3 * C], f32, kind="ExternalInput")
    idx_d = nc.dram_tensor("idx", [128, IC], mybir.dt.int16, kind="ExternalInput")
    deg_d = nc.dram_tensor("deg", [P, T], f32, kind="ExternalInput")
    msk_d = nc.dram_tensor("msk", [128, 2 * R], bf16, kind="ExternalInput")
    out_d = nc.dram_tensor("out", [PC, C], f32, kind="ExternalOutput")

    with tile.TileContext(nc) as tc:
        with (
            tc.tile_pool(name="cst", bufs=1) as cst,
            tc.tile_pool(name="sb", bufs=3) as sb,
            tc.tile_pool(name="stg", bufs=5) as stg,
            tc.tile_pool(name="selp", bufs=4) as selp,
            tc.tile_pool(name="psm", bufs=3, space="PSUM") as psm,
            tc.tile_pool(name="pst", bufs=2, space="PSUM") as pst,
            tc.tile_pool(name="acc", bufs=3, space="PSUM") as accp,
            tc.tile_pool(name="dram", bufs=1, space="DRAM") as dram,
        ):
            ident = cst.tile([P, P], f32)
            make_identity(nc, ident[:])
            identb = cst.tile([P, P], bf16)
            make_identity(nc, identb[:])
            w1 = cst.tile([FIN + 1, 3 * H], bf16)
            nc.sync.dma_start(out=w1[:], in_=w1_d[:])
            XC = 7  # x feature-chunks of 7 tiles each
            xc = []
            for k in range(XC):
                xck = cst.tile([FIN + 1, (T // XC) * P], bf16, tag=f"xc{k}",
                               name=f"xc{k}")
                nc.sync.dma_start(
                    out=xck[:],
                    in_=xt_d[:, k * (T // XC) * P:(k + 1) * (T // XC) * P])
                xc.append(xck)
            mskc = cst.tile([128, 2 * R], bf16)
            nc.sync.dma_start(out=mskc[:], in_=msk_d[:])
            w2 = cst.tile([H + 1, 3 * C], f32)
            nc.sync.dma_start(out=w2[:], in_=w2_d[:])
            idx = cst.tile([128, IC], mybir.dt.int16)
            nc.sync.dma_start(out=idx[:], in_=idx_d[:])
            deg = cst.tile([P, T], f32)
            nc.sync.dma_start(out=deg[:], in_=deg_d[:])
            M = mybir.AluOpType.mult
            # dinv = (deg>0) / sqrt(max(deg,1))
            dinv = cst.tile([P, T], f32)
            dinv2 = cst.tile([P, T], f32)
            tmpd = cst.tile([P, T], f32)
            nc.vector.tensor_scalar(out=tmpd[:], in0=deg[:], scalar1=1.0, scalar2=None,
                                    op0=mybir.AluOpType.max)
            nc.vector.reciprocal(out=tmpd[:], in_=tmpd[:])
            nc.scalar.sqrt(out=tmpd[:], in_=tmpd[:])
            mk = cst.tile([P, T], f32)
            nc.vector.tensor_scalar(out=mk[:], in0=deg[:], scalar1=0.5, scalar2=None,
                                    op0=mybir.AluOpType.is_ge)
            nc.vector.tensor_mul(out=dinv[:], in0=tmpd[:], in1=mk[:])
            nc.vector.tensor_mul(out=dinv2[:], in0=dinv[:], in1=dinv[:])
            negd = cst.tile([P, T], f32)
            negd2 = cst.tile([P, T], f32)
            nc.vector.tensor_scalar(out=negd[:], in0=dinv[:], scalar1=-1.0,
                                    scalar2=None, op0=M)
            nc.vector.tensor_scalar(out=negd2[:], in0=dinv2[:], scalar1=-1.0,
                                    scalar2=None, op0=M)

            vbuf = cst.tile([P, T, H], f32)
            t0buf = cst.tile([P, T, H], f32)
            v2buf = cst.tile([P, T, C], f32)
            t02buf = cst.tile([P, T, C], f32)

            agin = [dram.tile([PC, H], bf16, tag=f"agin{i}", name=f"agin{i}")
                    for i in range(4)]
            tabs = [dram.tile([NC * PC, H], bf16, addr_space="Shared", tag=f"tab{i}",
                              name=f"tab{i}") for i in range(4)]

            def allgather(i):
                nc.gpsimd.collective_compute(
                    "AllGather", mybir.AluOpType.bypass,
                    replica_groups=[list(range(NC))],
                    ins=[agin[i][:].opt()], outs=[tabs[i][:].opt()])

            def agwrite(i, t, src):
                nc.sync.dma_start(out=agin[i][t * P:(t + 1) * P, :], in_=src[:])

            dc = lambda t: dinv[:, t:t + 1]
            ndc = lambda t: negd[:, t:t + 1]
            nd2c = lambda t: negd2[:, t:t + 1]

            def spmv(i, epilogue, ag_target=None):
                off = 0
                for g, bt in enumerate(batches):
                    rb = sum(int(da[t]) for t in bt)
                    colbase = 8 * off
                    stA = stg.tile([P, rb, 2 * H], bf16, tag="stA", name="stA")
                    nc.gpsimd.dma_gather(
                        out_ap=stA[:],
                        in_ap=tabs[i][:].rearrange("(u a) f -> u (a f)", a=2),
                        idxs_ap=idx[:, colbase:colbase + 8 * rb],
                        num_idxs=rb * P, num_idxs_reg=rb * P,
                        elem_size=2 * H, single_packet=False, queue_num=g % 4)
                    mL = mskc[:, off:off + rb].rearrange(
                        "p (r o) -> p r o", o=1).broadcast_to([P, rb, H])
                    mR = mskc[:, R + off:R + off + rb].rearrange(
                        "p (r o) -> p r o", o=1).broadcast_to([P, rb, H])
                    nc.vector.tensor_mul(out=stA[:, :, 0:H], in0=stA[:, :, 0:H],
                                         in1=mL)
                    nc.vector.tensor_mul(out=stA[:, :, H:2 * H],
                                         in0=stA[:, :, H:2 * H], in1=mR)
                    sel = selp.tile([P, rb, H], bf16, tag="sel", name="sel")
                    nc.vector.tensor_add(out=sel[:], in0=stA[:, :, 0:H],
                                         in1=stA[:, :, H:2 * H])
                    ao = 0
                    for t in bt:
                        dt = int(da[t])
                        p512 = accp.tile([P, 8, H], f32, tag="acc", space="PSUM",
                                         name="p512")
                        chunks = [(sc, min(8, dt - sc)) for sc in range(0, dt, 8)]
                        cov = chunks[0][1]
                        for k, (sc, r) in enumerate(chunks):
                            nc.tensor.matmul(out=p512[:, 0:r, :], lhsT=identb[:],
                                             rhs=sel[:, ao + sc:ao + sc + r, :],
                                             start=(k == 0),
                                             stop=(k == len(chunks) - 1))
                        acc = sb.tile([P, H], f32, tag="accs", name="accs")
                        nc.vector.tensor_reduce(
                            out=acc[:],
                            in_=p512[:, 0:cov, :].rearrange("p a b -> p b a"),
                            axis=mybir.AxisListType.X, op=mybir.AluOpType.add)
                        ao += dt
                        epilogue(t, acc)
                    off += rb
                if ag_target is not None:
                    allgather(ag_target)

            # ---- layer 1, z columns only (critical path to AG0) ----
            zbuf = cst.tile([P, T, H], bf16)
            for t in range(T):
                xck = xc[min(t // (T // XC), XC - 1)]
                xsl = xck[:, (t - min(t // (T // XC), XC - 1) * (T // XC)) * P:
                          (t - min(t // (T // XC), XC - 1) * (T // XC) + 1) * P]
                pm = psm.tile([P, H], f32, tag="mm", space="PSUM")
                nc.tensor.matmul(out=pm[:], lhsT=xsl,
                                 rhs=w1[:, 0:H], start=True, stop=True)
                nc.scalar.mul(out=zbuf[:, t, :], in_=pm[:], mul=dc(t))
            nc.sync.dma_start(
                out=agin[0][:].rearrange("(t p) f -> p t f", p=P), in_=zbuf[:])
            allgather(0)
            # ---- layer 1, v/t0 columns (overlaps spmv1 descriptor gen) ----
            for t in range(T):
                xck = xc[min(t // (T // XC), XC - 1)]
                xsl = xck[:, (t - min(t // (T // XC), XC - 1) * (T // XC)) * P:
                          (t - min(t // (T // XC), XC - 1) * (T // XC) + 1) * P]
                pm = psm.tile([P, 2 * H], f32, tag="mm", space="PSUM")
                nc.tensor.matmul(out=pm[:], lhsT=xsl,
                                 rhs=w1[:, H:3 * H], start=True, stop=True)
                nc.vector.tensor_copy(out=vbuf[:, t, :], in_=pm[:, 0:H])
                nc.vector.tensor_copy(out=t0buf[:, t, :], in_=pm[:, H:2 * H])

            # ---- spmv1 -> mhat -> AG1 (chunks fired inside spmv) ----
            def epi1(t, acc):
                mh = sb.tile([P, H], bf16, tag="mh", name="mh")
                tv = sb.tile([P, H], f32, tag="tv", name="tv")
                nc.scalar.mul(out=tv[:], in_=vbuf[:, t, :], mul=dc(t))
                # mh = nd2*acc + dinv*v
                mh2 = sb.tile([P, H], f32, tag="mh2", name="mh2")
                nc.scalar.mul(out=mh2[:], in_=acc[:], mul=nd2c(t))
                nc.vector.tensor_add(out=mh[:], in0=mh2[:], in1=tv[:])
                agwrite(1, t, mh)
            spmv(0, epi1, ag_target=1)

            # ---- spmv2 -> h = relu(t0 - dinv*s) -> fused layer-2 matmuls ----
            def epi2(t, acc):
                hb = sb.tile([P, H], f32, tag="hb", name="hb")
                nc.scalar.mul(out=hb[:], in_=acc[:], mul=ndc(t))
                nc.vector.tensor_add(out=hb[:], in0=hb[:], in1=t0buf[:, t, :])
                nc.vector.tensor_scalar(out=hb[:], in0=hb[:],
                                        scalar1=0.0, scalar2=None,
                                        op0=mybir.AluOpType.max)
                pt = pst.tile([H, P], f32, tag="tr", space="PSUM")
                nc.tensor.transpose(out=pt[:], in_=hb[:], identity=ident[:])
                ht = sb.tile([H + 1, P], f32, tag="ht")
                nc.vector.tensor_copy(out=ht[0:H, :], in_=pt[:])
                nc.vector.memset(ht[H:H + 1, :], 1.0)
                pm = psm.tile([P, 3 * C], f32, tag="mm", space="PSUM")
                nc.tensor.matmul(out=pm[:], lhsT=ht[:], rhs=w2[:], start=True, stop=True)
                z2 = sb.tile([P, H], bf16, tag="z2")
                nc.vector.memset(z2[:, C:H], 0.0)
                nc.scalar.mul(out=z2[:, 0:C], in_=pm[:, 0:C], mul=dc(t))
                agwrite(2, t, z2)
                nc.vector.tensor_copy(out=v2buf[:, t, :], in_=pm[:, C:2 * C])
                nc.vector.tensor_copy(out=t02buf[:, t, :], in_=pm[:, 2 * C:3 * C])
            spmv(1, epi2, ag_target=2)

            # ---- spmv3 -> mhat2 -> AG3 ----
            def epi3(t, acc):
                m2 = sb.tile([P, H], bf16, tag="m2", name="m2")
                nc.vector.memset(m2[:, C:H], 0.0)
                m2f = sb.tile([P, C], f32, tag="m2f", name="m2f")
                nc.scalar.mul(out=m2f[:], in_=acc[:, 0:C], mul=nd2c(t))
                tv = sb.tile([P, C], f32, tag="tv2", name="tv")
                nc.scalar.mul(out=tv[:], in_=v2buf[:, t, :], mul=dc(t))
                nc.vector.tensor_add(out=m2[:, 0:C], in0=m2f[:], in1=tv[:])
                agwrite(3, t, m2)
            spmv(2, epi3, ag_target=3)

            # ---- spmv4 -> logits; log_softmax in two halves so the first
            # half overlaps the last gather batches ----
            SP = 40
            lgA = cst.tile([P, SP, C], f32)
            lgB = cst.tile([P, T - SP, C], f32)
            def epi4(t, acc):
                lg = sb.tile([P, C], f32, tag="lg", name="lg")
                nc.scalar.mul(out=lg[:], in_=acc[:, 0:C], mul=ndc(t))
                dst = lgA[:, t, :] if t < SP else lgB[:, t - SP, :]
                nc.vector.tensor_add(out=dst, in0=lg[:], in1=t02buf[:, t, :])
            spmv(3, epi4)
            for lgall, t0_, t1_ in ((lgA, 0, SP), (lgB, SP, T)):
                TT = t1_ - t0_
                nmx = cst.tile([P, TT], f32, tag=f"nmx{t0_}", name=f"nmx{t0_}")
                nc.vector.tensor_reduce(out=nmx[:], in_=lgall[:],
                                        axis=mybir.AxisListType.X,
                                        op=mybir.AluOpType.max, negate=True)
                nc.vector.tensor_add(
                    out=lgall[:], in0=lgall[:],
                    in1=nmx[:].rearrange("p (t o) -> p t o", o=1)
                    .broadcast_to([P, TT, C]))
                exall = cst.tile([P, TT, C], f32, tag=f"ex{t0_}", name=f"ex{t0_}")
                nc.scalar.activation(out=exall[:], in_=lgall[:],
                                     func=mybir.ActivationFunctionType.Exp)
                sm = cst.tile([P, TT], f32, tag=f"sm{t0_}", name=f"sm{t0_}")
                nc.vector.tensor_reduce(out=sm[:], in_=exall[:],
                                        axis=mybir.AxisListType.X,
                                        op=mybir.AluOpType.add)
                rs = cst.tile([P, TT], f32, tag=f"rs{t0_}", name=f"rs{t0_}")
                nc.vector.reciprocal(out=rs[:], in_=sm[:])
                nls = cst.tile([P, TT], f32, tag=f"nls{t0_}", name=f"nls{t0_}")
                nc.scalar.activation(out=nls[:], in_=rs[:],
                                     func=mybir.ActivationFunctionType.Ln)
                nc.vector.tensor_add(
                    out=exall[:], in0=lgall[:],
                    in1=nls[:].rearrange("p (t o) -> p t o", o=1)
                    .broadcast_to([P, TT, C]))
                nc.sync.dma_start(
                    out=out_d[t0_ * P:t1_ * P, :]
                    .rearrange("(t p) c -> p t c", p=P), in_=exall[:])
    nc.compile()
    return nc


def kernel(x, edge_index, W1, b1, W2, b2):
    x = np.asarray(x, np.float32)
    W1 = np.asarray(W1, np.float32); b1 = np.asarray(b1, np.float32)
    W2 = np.asarray(W2, np.float32); b2 = np.asarray(b2, np.float32)

    order, da, batches, idxbuf, maskbuf, deg_pt, xts = _preprocess(x, edge_index)

    import ml_dtypes
    w1p = np.zeros((FIN + 1, 3 * H), ml_dtypes.bfloat16)
    w1p[:FIN, 0:H] = 2.0 * W1[2]
    w1p[:FIN, H:2 * H] = W1[1]
    w1p[:FIN, 2 * H:3 * H] = W1[0] - W1[2]
    w1p[FIN, 2 * H:3 * H] = b1
    w2p = np.zeros((H + 1, 3 * C), np.float32)
    w2p[:H, 0:C] = 2.0 * W2[2]
    w2p[:H, C:2 * C] = W2[1]
    w2p[:H, 2 * C:3 * C] = W2[0] - W2[2]
    w2p[H, 2 * C:3 * C] = b2

    trace = bool(os.environ.get("CHEB_TRACE"))
    if trace:
        import sys, types
        try:
            from trn_agent_boot.trn_boot import _ntff_profile_via_ctypes
            m = types.ModuleType("antenv.axon_hooks")
            m.get_axon_ntff_profile_hook = (
                lambda: _ntff_profile_via_ctypes("/opt/axon/libaxon_pjrt.so"))
            sys.modules["antenv.axon_hooks"] = m
        except Exception:
            trace = False

    nc = _build(da, batches)
    from concourse.bass_utils import run_bass_kernel_spmd
    ins = [{"xt": xts[c], "w1p": w1p, "w2p": w2p, "idx": idxbuf[c],
            "deg": deg_pt[c], "msk": maskbuf[c]} for c in range(NC)]
    res = run_bass_kernel_spmd(nc, ins, core_ids=list(range(NC)), trace=trace)
    if trace and res.exec_time_ns is not None:
        print(f"HW exec time: {res.exec_time_ns} ns")

    out = np.empty((N, C), np.float32)
    for c in range(NC):
        nq = min(PC, (N - c + NC - 1) // NC)
        nodes = order[np.arange(nq) * NC + c]
        out[nodes] = res.results[c]["out"][:nq]
    return out
